# revision 53
# baseline (speedup 1.0000x reference)
"""Trainium2 Bass kernel for the masked style-attention module.

Shapes (hardcoded): B=4, C_IN=256, C_KEY=448, H=W=64, N=4096.
Sharding: 8 cores = batch (4) x query-row half (2). Each core computes
out[b][:, h*2048:(h+1)*2048] in [c, n] layout (c = ct*128 + partition).

Key restructure vs v0 (333us -> 293us):
  * Both 1x1-conv projections of the attention folded on the host:
    W' = Wf^T @ Wg, so S[n,m] = ckey_n . W' . skey_m + u[m] + v[n] + c.
    v[n]/const drop (softmax over m is row-invariant); u = skey^T(Wg^T bf)
    and the mask ride as extra contraction rows (k-tiles {128,128,128,66}).
    The device projects only the SMALL side: fq = W'^T @ ckey (2048 cols);
    the augmented skey is the T-matmul's stationary operand via DMA.
  * PV matmul swapped: lhsT = hv2 (values), moving = P. Accumulators land
    in [c, n] layout: mvn scale/bias become per-partition ops, natural
    output layout, no transposes/broadcast tiles.
  * acc banks double-buffered by packing two accumulation groups per PSUM
    bank (start=False + DVE pre-zero) -- the whole R/finalize chain gets a
    full chunk of slack, so its latency never stalls the PE.
  * R = sum_m P via GPSIMD running adds (absorbed lag) then a single
    all-ones fp32 matmul = partition-sum + broadcast in one op; DVE
    reciprocal on the broadcast [128,nw] tile (fast path).
  * ACT stays Exp-only during attention (function-table reloads cost
    1.3us): mvn, a_t = rsqrt via DVE magic-Newton; H squares on GPSIMD.
  * mvn'd content precomputed up front; last two chunks are 128 wide so
    the closing serial finalize chain runs on small tiles.
"""

from contextlib import ExitStack

import numpy as np
import ml_dtypes

import concourse.bass as bass
from concourse import bacc
import concourse.mybir as mybir
import concourse.tile as tile
from concourse.bass_utils import run_bass_kernel_spmd

AF = mybir.ActivationFunctionType
ALU = mybir.AluOpType
F32 = mybir.dt.float32
BF16 = mybir.dt.bfloat16
U32 = mybir.dt.uint32

B, C_IN, C_KEY = 4, 256, 448
N = 4096
HALF = 2048
NEG = -1e15
EPS = 1e-5
CORR = N / (N - 1.0)  # unbiased-variance correction for mvn
MAGIC = 0x5F3759DF

KW = [128, 128, 128, 64]  # contraction tiles over 448 (W'^T @ ckey)
CO448 = [(0, 128), (128, 128), (256, 128), (384, 64)]  # fq output rows
KT = [128, 128, 128, 66]  # contraction tiles over 450 (T matmul, aug rows)


def _build():
    nc = bacc.Bacc("TRN2", target_bir_lowering=False)

    w4 = nc.dram_tensor("w4", [128, 4, 448], BF16, kind="ExternalInput")
    skaug = nc.dram_tensor("skaug", [128, 4, N], BF16, kind="ExternalInput")
    wh = nc.dram_tensor("wh", [128, 2, 256], BF16, kind="ExternalInput")
    bh = nc.dram_tensor("bh", [1, 256], F32, kind="ExternalInput")
    styl = nc.dram_tensor("styl", [128, 2, N], BF16, kind="ExternalInput")
    cont = nc.dram_tensor("cont", [128, 2, N], F32, kind="ExternalInput")
    ckey = nc.dram_tensor("ckey", [128, 4, HALF], BF16, kind="ExternalInput")
    cmrow = nc.dram_tensor("cmrow", [1, HALF], BF16, kind="ExternalInput")
    out_d = nc.dram_tensor("out", [128, 2, HALF], F32, kind="ExternalOutput")

    with tile.TileContext(nc, pool_alloc_mode="queue") as tc:
        with tc.tile_pool(name="persist", bufs=1) as persist:
            # T-matmul stationary side: host-augmented skey (rows 448 = u,
            # 449 = NEG*smi), DMA'd straight in -- no device projection of
            # the 4096-wide style side at all.
            skg = [
                persist.tile([KT[3] if k == 3 else 128, N], BF16, tag=f"g{k}", name=f"g{k}")
                for k in range(4)
            ]
            # moving side: fq = W'^T @ ckey computed on device (2048 cols),
            # plus aug rows 448 = ones, 449 = cm.
            fq = [
                persist.tile(
                    [KT[3] if k == 3 else 128, HALF], BF16, tag=f"fq{k}", name=f"fq{k}"
                )
                for k in range(4)
            ]
            nc.vector.memset(fq[3][64:65, :], 1.0)
            hv2 = persist.tile([128, 32, 512], BF16, tag="hv2", name="hv2")
            cx_sb = persist.tile([128, 2, N], F32, tag="cx_sb", name="cx_sb")
            ck_sb = persist.tile([128, 4, HALF], BF16, tag="ck_sb", name="ck_sb")
            a_t = persist.tile([128, 2], F32, tag="a_t", name="a_t")
            b_t = persist.tile([128, 2], F32, tag="b_t", name="b_t")
            bhb = persist.tile([128, 256], F32, tag="bhb", name="bhb")
            ones1 = persist.tile([1, 128], F32, tag="ones1", name="ones1")
            nc.vector.memset(ones1, 1.0)
            onesq = persist.tile([128, 128], BF16, tag="onesq", name="onesq")
            nc.vector.memset(onesq, 1.0)
            eps_t = persist.tile([128, 1], F32, tag="eps", name="eps")
            nc.vector.memset(eps_t, EPS)
            mgc = persist.tile([128, 512], U32, tag="mgc", name="mgc")
            nc.vector.memset(mgc, MAGIC)

            _pp = ExitStack()
            pw = _pp.enter_context(tc.tile_pool(name="pw", bufs=1))
            pin = _pp.enter_context(tc.tile_pool(name="pin", bufs=1))
            # DMA issue order = need order: G3 inputs first (w3 + skey
            # quarters), then H inputs, then cont (stats during G3/H),
            # then attention inputs.
            bh1 = pw.tile([1, 256], F32, tag="bh1", name="bh1")
            nc.sync.dma_start(bh1, bh[:, :])
            w4_t = pw.tile([128, 4, 448], BF16, tag="w4_t", name="w4_t")
            for k in range(4):
                nc.sync.dma_start(w4_t[:, k, :], w4[:, k, :])
            for k in range(4):
                for q in range(2):
                    nc.sync.dma_start(
                        ck_sb[:, k, q * 1024 : (q + 1) * 1024],
                        ckey[:, k, q * 1024 : (q + 1) * 1024],
                    )
            nc.sync.dma_start(fq[3][65:66, :], cmrow[:, :])
            wh_t = pw.tile([128, 2, 256], BF16, tag="wh_t", name="wh_t")
            nc.sync.dma_start(wh_t, wh[:, :, :])
            st_sb = pin.tile([128, 2, N], BF16, tag="st_sb", name="st_sb")
            nc.sync.dma_start(st_sb[:, :, 0:HALF], styl[:, :, 0:HALF])
            nc.sync.dma_start(st_sb[:, :, HALF:N], styl[:, :, HALF:N])
            # T stationary side streams straight to SBUF
            for k in range(4):
                for q in range(2):
                    nc.sync.dma_start(
                        skg[k][0 : KT[k], q * HALF : (q + 1) * HALF],
                        skaug[0 : KT[k], k, q * HALF : (q + 1) * HALF],
                    )
            nc.sync.dma_start(cx_sb[:, 0, :], cont[:, 0, :])
            nc.sync.dma_start(cx_sb[:, 1, :], cont[:, 1, :])

            # ---- bhb broadcast tile (PE 1-row matmul) ----
            with tc.tile_pool(name="psumB", bufs=1, space="PSUM") as ppb:
                pb = ppb.tile([128, 256], F32, tag="pbh", name="pbh")
                nc.tensor.matmul(
                    pb, lhsT=ones1[0:1, :], rhs=bh1[0:1, :], start=True, stop=True
                )
                nc.vector.tensor_copy(bhb, pb)

            # ---- Phase F' (= W'^T @ ckey, 2048 cols) interleaved with H ----
            with (
                tc.tile_pool(name="psumG", bufs=1, space="PSUM") as ppg,
                tc.tile_pool(name="psumH", bufs=4, space="PSUM") as pph,
            ):

                def fq_block(co_i):
                    co0, cosz = CO448[co_i]
                    pgs = [
                        ppg.tile([128, 512], F32, tag=f"pg{ch}", name=f"pg{ch}")
                        for ch in range(4)
                    ]
                    for k in range(4):
                        for ch in range(4):
                            csl = slice(ch * 512, (ch + 1) * 512)
                            nc.tensor.matmul(
                                pgs[ch][0:cosz, :],
                                lhsT=w4_t[0 : KW[k], k, co0 : co0 + cosz],
                                rhs=ck_sb[0 : KW[k], k, csl],
                                start=(k == 0),
                                stop=(k == 3),
                            )
                    for ch in range(4):
                        csl = slice(ch * 512, (ch + 1) * 512)
                        nc.scalar.copy(fq[co_i][0:cosz, csl], pgs[ch][0:cosz, :])

                def h_block(mt0, mt1):
                    for mt in range(mt0, mt1):
                        ph = pph.tile([128, 256], F32, tag="ph", name="ph")
                        msl = slice(mt * 128, (mt + 1) * 128)
                        for k in range(2):
                            nc.tensor.matmul(
                                ph,
                                lhsT=st_sb[:, k, msl],
                                rhs=wh_t[:, k, :],
                                start=(k == 0),
                                stop=(k == 1),
                            )
                        nc.vector.tensor_add(hv2[:, mt, 0:256], ph, bhb)
                        nc.gpsimd.tensor_mul(
                            hv2[:, mt, 256:512], hv2[:, mt, 0:256], hv2[:, mt, 0:256]
                        )

                for co_i in range(4):
                    fq_block(co_i)
                    h_block(co_i * 8, (co_i + 1) * 8)

            # ---- mvn stats (DVE/ACT, overlaps G3/H) ----
            pm = _pp.enter_context(tc.tile_pool(name="mvn", bufs=1))
            mvs = []
            for ct in range(2):
                stats = pm.tile([128, 8, 6], F32, tag=f"stats{ct}", name=f"stats{ct}")
                for i in range(8):
                    nc.vector.bn_stats(
                        out=stats[:, i, :], in_=cx_sb[:, ct, i * 512 : (i + 1) * 512]
                    )
                mv = pm.tile([128, 2], F32, tag=f"mv{ct}", name=f"mv{ct}")
                nc.vector.bn_aggr(out=mv, in_=stats)
                mvs.append(mv)
            # a_t = 1/sqrt(var*CORR + EPS) via DVE fast-rsqrt + one Newton
            # step: no Ln/Exp on ACT, so no function-table reloads before
            # the attention exp stream starts.
            for ct in range(2):
                c1 = slice(ct, ct + 1)
                vc = pm.tile([128, 2], F32, tag="vc", name="vc")
                nc.vector.tensor_scalar(
                    vc[:, c1], mvs[ct][:, 1:2], CORR, EPS, ALU.mult, ALU.add
                )
                shs = pm.tile([128, 2], U32, tag="shs", name="shs")
                nc.vector.tensor_scalar(
                    shs[:, c1], vc.bitcast(U32)[:, c1], 1, None,
                    ALU.logical_shift_right,
                )
                ys = pm.tile([128, 2], F32, tag="ys", name="ys")
                nc.vector.tensor_tensor(
                    out=ys.bitcast(U32)[:, c1], in0=mgc[:, 0:1], in1=shs[:, c1],
                    op=ALU.subtract,
                )
                tas = pm.tile([128, 2], F32, tag="tas", name="tas")
                nc.vector.tensor_mul(tas[:, c1], ys[:, c1], ys[:, c1])
                tbs = pm.tile([128, 2], F32, tag="tbs", name="tbs")
                nc.vector.tensor_mul(tbs[:, c1], tas[:, c1], vc[:, c1])
                tcs = pm.tile([128, 2], F32, tag="tcs", name="tcs")
                nc.vector.tensor_scalar(
                    tcs[:, c1], tbs[:, c1], -0.5, 1.5, ALU.mult, ALU.add
                )
                nc.vector.tensor_mul(a_t[:, c1], ys[:, c1], tcs[:, c1])
                nc.vector.scalar_tensor_tensor(
                    out=b_t[:, c1],
                    in0=mvs[ct][:, 0:1],
                    scalar=-1.0,
                    in1=a_t[:, c1],
                    op0=ALU.mult,
                    op1=ALU.mult,
                )
            # mvn'd content for this core's half, precomputed on DVE (per-
            # partition AP scalars) so attention-time finalize never blocks
            # the ACT exp stream
            mvnall = persist.tile([128, 2, HALF], BF16, tag="mvnall", name="mvnall")
            for ct in range(2):
                nc.vector.tensor_scalar(
                    mvnall[:, ct, :],
                    cx_sb[:, ct, 0:HALF],
                    a_t[:, ct : ct + 1],
                    b_t[:, ct : ct + 1],
                    ALU.mult,
                    ALU.add,
                )
            _pp.close()  # free projection inputs + stats SBUF

            # ---- Phase D: attention ----
            _dpools = ExitStack()
            fin = _dpools.enter_context(tc.tile_pool(name="fin", bufs=1))
            ptp = _dpools.enter_context(tc.tile_pool(name="ptp", bufs=1))
            prr = _dpools.enter_context(tc.tile_pool(name="prr", bufs=1))
            with (
                tc.tile_pool(name="ppt", bufs=3, space="PSUM") as ppt,
                tc.tile_pool(name="ppacc", bufs=2, space="PSUM") as ppacc,
                tc.tile_pool(name="ppr", bufs=1, space="PSUM") as ppr,
            ):
                pend = {}

                def fin_a(st):
                    """R partition sum + broadcast in one all-ones matmul.
                    racc quantized to bf16 first: a bf16 matmul is 4x faster
                    than fp32 and the R error (~0.4%/sqrt(128)) is noise."""
                    nw = st["nw"]
                    rb = fin.tile([128, 256], BF16, tag="rb", name="rb", bufs=2)
                    nc.vector.tensor_copy(rb[:, 0:nw], st["racc"][:, 0:nw])
                    prb = ppr.tile([128, 256], F32, tag="prb", name="prb")
                    nc.tensor.matmul(
                        prb[:, 0:nw],
                        lhsT=onesq,
                        rhs=rb[:, 0:nw],
                        start=True,
                        stop=True,
                    )
                    st["prb"] = prb

                def fin_b(st):
                    """Reciprocal + divide out mean/m2."""
                    acc, nw = st["acc"], st["nw"]
                    rinvS = fin.tile([128, 256], F32, tag="rinvS", name="rinvS", bufs=2)
                    nc.vector.reciprocal(rinvS[:, 0:nw], st["prb"][:, 0:nw])
                    meanS = fin.tile(
                        [128, 2, 256], F32, tag="meanS", name="meanS", bufs=2
                    )
                    m2S = fin.tile([128, 2, 256], F32, tag="m2S", name="m2S", bufs=2)
                    for ci in range(2):
                        csl = slice(ci * 256, ci * 256 + nw)
                        nc.vector.tensor_mul(
                            meanS[:, ci, 0:nw], acc[0][:, csl], rinvS[:, 0:nw]
                        )
                        nc.vector.tensor_mul(
                            m2S[:, ci, 0:nw], acc[1][:, csl], rinvS[:, 0:nw]
                        )
                    st["meanS"], st["m2S"] = meanS, m2S

                def fin_rest(st):
                    """std = sqrt(relu(m2 - mean^2)) via fast-rsqrt + one
                    Newton step on DVE (ACT must stay Exp-only: function
                    table reloads cost 1.3us and stall the exp stream).
                    Issued as two independent per-ct chains, interleaved so
                    each op's dependency is two slots back -- hides the
                    ~0.3-0.6us inter-op semaphore latency in the tail."""
                    nofs, nw = st["nofs"], st["nw"]
                    meanS, m2S = st["meanS"], st["m2S"]
                    mgc3 = mgc.rearrange("p (a b) -> p a b", a=2)
                    msq = fin.tile([128, 2, 256], F32, tag="fw", name="msq", bufs=6)
                    varp = fin.tile([128, 2, 256], F32, tag="fw", name="varp", bufs=6)
                    varc = fin.tile([128, 2, 256], F32, tag="fw", name="varc", bufs=6)
                    sh = fin.tile([128, 2, 256], U32, tag="fw", name="sh", bufs=6)
                    y = fin.tile([128, 2, 256], F32, tag="fw", name="y0", bufs=6)
                    ta = fin.tile([128, 2, 256], F32, tag="fw", name="ta", bufs=6)
                    o2 = fin.tile([128, 2, 256], F32, tag="o2", name="o2", bufs=2)
                    shu = varc.bitcast(U32)
                    yu = y.bitcast(U32)

                    def ops(ct):
                        s = (slice(None), ct, slice(0, nw))
                        mean_c, m2_c = meanS[s], m2S[s]
                        yield lambda: nc.vector.tensor_mul(msq[s], mean_c, mean_c)
                        yield lambda: nc.vector.tensor_sub(varp[s], m2_c, msq[s])
                        yield lambda: nc.vector.tensor_scalar_max(
                            varc[s], varp[s], 1e-20
                        )
                        yield lambda: nc.vector.tensor_scalar(
                            sh[s], shu[s], 1, None, ALU.logical_shift_right
                        )
                        yield lambda: nc.vector.tensor_tensor(
                            out=yu[s], in0=mgc3[s], in1=sh[s], op=ALU.subtract
                        )
                        yield lambda: nc.vector.tensor_mul(ta[s], y[s], y[s])
                        yield lambda: nc.vector.tensor_mul(ta[s], ta[s], varc[s])
                        yield lambda: nc.vector.tensor_scalar(
                            ta[s], ta[s], -0.5, 1.5, ALU.mult, ALU.add
                        )
                        yield lambda: nc.vector.tensor_mul(y[s], y[s], ta[s])
                        yield lambda: nc.vector.tensor_mul(varc[s], varc[s], y[s])
                        yield lambda: nc.vector.tensor_mul(
                            varc[s],
                            mvnall[:, ct, nofs : nofs + nw],
                            varc[s],
                        )
                        yield lambda: nc.vector.tensor_add(o2[s], varc[s], mean_c)

                    for opa, opb in zip(ops(0), ops(1)):
                        opa()
                        opb()
                    nc.sync.dma_start(out_d[:, :, nofs : nofs + nw], o2[:, :, 0:nw])

                def mm2(st, mt, pt_ap, nw):
                    # ci 0/1 = Hv -> acc[0] halves; ci 2/3 = Hv^2 -> acc[1]
                    for ci in range(4):
                        csl = slice((ci % 2) * 256, (ci % 2) * 256 + nw)
                        nc.tensor.matmul(
                            st[ci // 2][:, csl],
                            lhsT=hv2[:, mt, ci * 128 : (ci + 1) * 128],
                            rhs=pt_ap,
                            start=False,
                            stop=(mt == 31),
                            skip_group_check=True,
                        )

                # 7 chunks of 256 query cols then 128/64/64: the last
                # finalize chains (serial DVE, ~6ns/elem) run on small tiles
                # so the kernel tail stays short.
                chunks = [(i * 256, 256) for i in range(7)] + [(1792, 128), (1920, 128)]
                prevs = []
                for hc, (nofs, nw) in enumerate(chunks):
                    nsl = slice(nofs, nofs + nw)
                    # two PSUM banks per chunk, each holding TWO accumulation
                    # groups (cols 0:256 / 256:512). No start=True: banks are
                    # pre-zeroed by DVE so the groups can share a zero-region.
                    acc = [
                        ppacc.tile([128, 512], F32, tag=f"accp{j}", name=f"accp{j}")
                        for j in range(2)
                    ]
                    for j in range(2):
                        nc.vector.memset(acc[j], 0.0)
                    racc = prr.tile([128, 256], F32, tag="racc", name="racc", bufs=2)

                    for mt in range(32):
                        msl = slice(mt * 128, (mt + 1) * 128)
                        tp = ppt.tile([128, 256], F32, tag="tp", name="tp")
                        for k in range(4):
                            nc.tensor.matmul(
                                tp[:, 0:nw],
                                lhsT=skg[k][0 : KT[k], msl],
                                rhs=fq[k][0 : KT[k], nsl],
                                start=(k == 0),
                                stop=(k == 3),
                            )
                        pt = ptp.tile([128, 256], BF16, tag="pt", name="pt", bufs=4)
                        nc.scalar.activation(pt[:, 0:nw], tp[:, 0:nw], AF.Exp)
                        # R running sum: GPSIMD mid-stream (its steady-state
                        # lag is absorbed by the double-buffered acc banks),
                        # DVE for the last chunk to keep the tail short.
                        reng = nc.vector if hc == len(chunks) - 1 else nc.gpsimd
                        if mt == 0:
                            reng.tensor_copy(racc[:, 0:nw], pt[:, 0:nw])
                        else:
                            reng.tensor_add(
                                racc[:, 0:nw], racc[:, 0:nw], pt[:, 0:nw]
                            )
                        prevs.append((acc, mt, pt[:, 0:nw], nw))
                        # 2-deep exp lag; drain the carried PV groups in the
                        # first slot so acc banks close before fin_b
                        drain = 1 if mt == 0 else 2
                        while len(prevs) > drain:
                            mm2(*prevs.pop(0))
                        if mt == 0 and pend:
                            fin_a(pend)
                        if mt == 2 and pend:
                            fin_b(pend)
                        if mt == 4 and pend:
                            fin_rest(pend)
                            pend.clear()
                    pend = {"nofs": nofs, "nw": nw, "acc": acc, "racc": racc}
                for pr in prevs:
                    mm2(*pr)
                fin_a(pend)
                fin_b(pend)
                fin_rest(pend)
            _dpools.close()
    nc.finalize()
    return nc


_nc_cache = None
last_results = None  # BassKernelResults of the most recent run (for test.py)


def _bf16(x):
    return np.asarray(x, dtype=ml_dtypes.bfloat16)


def _pad_k(a, ksz):
    """[sum(ksz), M] -> [128, len(ksz), M] zero-padded k-tiles."""
    m = a.shape[1]
    outp = np.zeros((128, len(ksz), m), a.dtype)
    r = 0
    for k, s in enumerate(ksz):
        outp[0:s, k, :] = a[r : r + s, :]
        r += s
    return np.ascontiguousarray(outp)


def prepare_in_maps(
    content,
    style,
    content_key,
    style_key,
    content_mask,
    style_mask,
    Wf,
    bf,
    Wg,
    bg,
    Wh,
    bh,
):
    f32 = np.float32
    # host-folded projections: W' = Wf^T @ Wg; u = skey^T (Wg^T bf)
    Wp = np.asarray(Wf, f32).T @ np.asarray(Wg, f32)  # [448, 448] rows=ci
    u_w = np.asarray(Wg, f32).T @ np.asarray(bf, f32)  # [448]
    w4_in = _pad_k(_bf16(Wp), KW)  # lhsT [ci, c] -> [128, 4, 448]
    whT = np.asarray(Wh, f32).T.reshape(2, 128, 256).transpose(1, 0, 2)
    wh_in = np.ascontiguousarray(_bf16(whT))
    bh_in = np.ascontiguousarray(np.asarray(bh, f32)[None, :])

    in_maps = []
    for c in range(8):
        b, h = divmod(c, 2)
        hsl = slice(h * HALF, (h + 1) * HALF)
        osl = slice((1 - h) * HALF, (2 - h) * HALF)
        sk = np.asarray(style_key[b], f32).reshape(C_KEY, N)
        ck = np.asarray(content_key[b], f32).reshape(C_KEY, N)[:, hsl]
        st = np.asarray(style[b], f32).reshape(C_IN, N)
        co = np.asarray(content[b], f32).reshape(C_IN, N)
        smi_in = (np.asarray(style_mask[b], np.int32).reshape(1, N) == 0).astype(f32)
        cm = (np.asarray(content_mask[b], np.int32).reshape(N)[hsl] != 0).astype(f32)
        # skey_aug k-tiles over 450: {sk[0:384], [sk[384:448]; u; NEG*smi]}
        u = sk.T @ u_w  # [N]
        sk_aug = np.concatenate(
            [sk, u[None, :], smi_in * np.float32(NEG)], 0
        )
        st_in = _bf16(st).reshape(2, 128, N).transpose(1, 0, 2)
        # cont permuted: this core's half first (stats invariant to order)
        co_p = np.concatenate([co[:, hsl], co[:, osl]], 1)
        cont_in = co_p.reshape(2, 128, N).transpose(1, 0, 2)
        in_maps.append(
            {
                "w4": w4_in,
                "skaug": _pad_k(_bf16(sk_aug), KT),
                "wh": wh_in,
                "bh": bh_in,
                "styl": np.ascontiguousarray(st_in),
                "cont": np.ascontiguousarray(cont_in),
                "ckey": _pad_k(_bf16(ck), KW),
                "cmrow": np.ascontiguousarray(_bf16(cm[None, :])),
            }
        )

    return in_maps


def get_nc():
    global _nc_cache
    if _nc_cache is None:
        _nc_cache = _build()
    return _nc_cache


def gather_output(outs):
    full = np.empty((B, C_IN, N), np.float32)
    for c in range(8):
        b, h = divmod(c, 2)
        o = np.asarray(outs[c])  # [128, 2, 2048]
        full[b][:, h * HALF : (h + 1) * HALF] = o.transpose(1, 0, 2).reshape(
            C_IN, HALF
        )
    return full.reshape(B, C_IN, 64, 64)


def kernel(**inputs):
    global last_results
    in_maps = prepare_in_maps(**inputs)
    res = run_bass_kernel_spmd(get_nc(), in_maps, core_ids=list(range(8)))
    last_results = res
    return gather_output([r["out"] for r in res.results])


if __name__ == "__main__":
    rng = np.random.default_rng(0)
    ins = {
        "content": rng.standard_normal((B, C_IN, 64, 64), dtype=np.float32),
        "style": rng.standard_normal((B, C_IN, 64, 64), dtype=np.float32),
        "content_key": rng.standard_normal((B, C_KEY, 64, 64), dtype=np.float32),
        "style_key": rng.standard_normal((B, C_KEY, 64, 64), dtype=np.float32),
        "content_mask": rng.integers(0, 2, (B, 1, 64, 64)).astype(np.int32),
        "style_mask": rng.integers(0, 2, (B, 1, 64, 64)).astype(np.int32),
        "Wf": (rng.standard_normal((C_KEY, C_KEY)) * 0.02).astype(np.float32),
        "bf": (rng.standard_normal((C_KEY,)) * 0.02).astype(np.float32),
        "Wg": (rng.standard_normal((C_KEY, C_KEY)) * 0.02).astype(np.float32),
        "bg": (rng.standard_normal((C_KEY,)) * 0.02).astype(np.float32),
        "Wh": (rng.standard_normal((C_IN, C_IN)) * 0.02).astype(np.float32),
        "bh": (rng.standard_normal((C_IN,)) * 0.02).astype(np.float32),
    }
    out = kernel(**ins)
    print("kernel output", out.shape, out.dtype, np.abs(out).mean())
